# revision 28
# baseline (speedup 1.0000x reference)
"""Trainium2 Bass kernel for the MAMGCN encoder block.

Strategy: data-parallel over batch B=16 across 8 NeuronCores (2 batches/core).
Host-side prep (untimed): shard x, repack small weights, pre-transpose x to
(t*64+f, n) layout, cast matmul operands to bf16. Device does everything else:
spatial attention (two fused weight matmuls -> product -> tanh-sigmoid ->
Vs@P -> exp -> column softmax), Chebyshev graph conv with Theta folded in
(Y = X @ Theta2 block-diag), all matmuls in bf16 with fp32 PSUM accumulation.

v3: T in 3 groups of 8 (conv matmuls stream one full 512-row PSUM bank),
Y-build interleaved with attention phases (tensor stays dense, HAM warm),
Y PSUM->SBUF copies split across DVE and ACT, one shared 4-slot PSUM pool
for Y-build + conv, x loads issued ahead of the attention-constant loads,
and the two batches software-pipelined so batch 1's attention/S phases
overlap batch 0's graph conv.
"""
import numpy as np
import ml_dtypes

B, N, F, T, K, FO = 16, 1024, 64, 24, 3, 64
NCORES = 8
BPC = B // NCORES          # batches per core
NCH = N // 128             # 8 partition chunks of N
NJ = (T * F) // 128        # 12 chunks of the tf dim
NG = 3                     # t-groups
TT = T // NG               # 8 t's per group
NJG = NJ // NG             # 4 tf-chunks per t-group
bf16 = ml_dtypes.bfloat16

_CACHE = {}


def _build_nc():
    import concourse.bacc as bacc
    import concourse.bass as bass
    import concourse.tile as tile
    import concourse.mybir as mybir

    fp32 = mybir.dt.float32
    bf = mybir.dt.bfloat16
    AF = mybir.ActivationFunctionType

    nc = bacc.Bacc(
        "TRN2", target_bir_lowering=False, debug=False,
        enable_asserts=True, num_devices=NCORES,
    )

    # ---- DRAM I/O ----
    x_d = nc.dram_tensor("x_tf", [BPC, NJ, 128, N], bf, kind="ExternalInput")
    bs_d = nc.dram_tensor("bs_t", [NCH, 128, N], bf, kind="ExternalInput")
    vs_d = nc.dram_tensor("vs_t", [NCH, 128, N], bf, kind="ExternalInput")
    cheb_d = nc.dram_tensor("cheb_t", [K, NCH, 128, N], bf, kind="ExternalInput")
    wcat_d = nc.dram_tensor("wcat", [NJ, 128, 2 * T], bf, kind="ExternalInput")
    th2_d = nc.dram_tensor("th2", [128, 2 * K * FO], bf, kind="ExternalInput")
    hrow_d = nc.dram_tensor("hrow", [NCH, 128, 1], fp32, kind="ExternalInput")
    # out[b, group, mchunk, p, o, tl]
    out_d = nc.dram_tensor("out", [BPC, NG, NCH, 128, FO, TT], bf,
                           kind="ExternalOutput")

    with tile.TileContext(nc) as tc:
        with (
            tc.tile_pool(name="const", bufs=1) as cpool,
            tc.tile_pool(name="work", bufs=2) as wpool,
            tc.tile_pool(name="big", bufs=1) as bpool,
            tc.tile_pool(name="ypool", bufs=2) as ypool,
            tc.tile_pool(name="psA", bufs=2, space="PSUM") as psA,
            tc.tile_pool(name="psB", bufs=6, space="PSUM") as psB,
        ):
            # ---- constants needed first (attention + Y-build) ----
            wcat_sb = cpool.tile([128, NJ, 2 * T], bf, tag="wcat")
            th2_sb = cpool.tile([128, 2 * K * FO], bf, tag="th2")
            vsT_sb = cpool.tile([128, NCH, N], bf, tag="vsT")
            bs_sb = cpool.tile([128, NCH, N], bf, tag="bs")
            hrow_sb = cpool.tile([128, NCH], fp32, tag="hrow")
            ones_sb = cpool.tile([128, 1], bf, tag="ones")
            one1_sb = cpool.tile([1, 1], fp32, tag="one1")
            nc.gpsimd.memset(ones_sb[:], 1.0)
            nc.gpsimd.memset(one1_sb[:], 1.0)

            def make_batch(b):
                st = {}

                def head():
                    st['x'] = x_sb = bpool.tile([128, NJ, N], bf, tag="x",
                                                name=f"x{b}")
                    st['e'] = bpool.tile([128, NCH, N], bf, tag="e",
                                         name=f"e{b}")
                    st['p'] = bpool.tile([128, NCH, N], bf, tag="p",
                                         name=f"p{b}")
                    st['a'] = bpool.tile([128, K, NCH, N], bf, tag="a",
                                         name=f"a{b}")
                    st['rT'] = bpool.tile([128, NCH], fp32, tag="rT",
                                          name=f"rT{b}")
                    st['ys'] = []
                    for j in range(NJ):
                        if b == 0:
                            nc.sync.dma_start(wcat_sb[:, j, :], wcat_d[j])
                        nc.sync.dma_start(x_sb[:, j, :], x_d[b, j])
                        if b == 0 and j == 0:
                            nc.sync.dma_start(th2_sb[:], th2_d[:])
                    if b == 0:
                        # attention constants follow the x load in queue order
                        for c in range(NCH):
                            nc.sync.dma_start(bs_sb[:, c, :], bs_d[c])
                        for c in range(NCH):
                            nc.sync.dma_start(vsT_sb[:, c, :], vs_d[c])
                            nc.sync.dma_start(hrow_sb[:, c:c + 1], hrow_d[c])
                    # attention pre-reductions (one pass over x)
                    st['att_c'] = att_c = wpool.tile([2 * T, N], bf,
                                                     tag="attc", bufs=1,
                                                     name=f"attc{b}")
                    st['att_r'] = att_r = wpool.tile([T, N], bf, tag="attr",
                                                     bufs=1, name=f"attr{b}")
                    pas = [psA.tile([2 * T, 512], fp32, tag="big",
                                    name=f"pa{b}_{s}") for s in range(2)]
                    for j in range(NJ):
                        for s in range(2):
                            nc.tensor.matmul(
                                pas[s][:, :],
                                wcat_sb[:, j, :],
                                x_sb[:, j, s * 512:(s + 1) * 512],
                                start=(j == 0), stop=(j == NJ - 1),
                            )
                    for s in range(2):
                        nc.scalar.copy(att_c[:, s * 512:(s + 1) * 512],
                                       pas[s][:])
                    # shift rows 24..47 down to partitions 0..23
                    nc.sync.dma_start(att_r[:], att_c[T:2 * T, :])

                def build_y(g):
                    x_sb = st['x']
                    y_sb = ypool.tile([128, NCH, K, TT, FO], bf, tag="y",
                                      name=f"y{b}_{g}")
                    st['ys'].append(y_sb)
                    for j in range(NJG * g, NJG * (g + 1)):
                        tl0 = 2 * (j - NJG * g)
                        for cn in range(NCH):
                            # py layout (k, par, o); one strided copy moves
                            # both t's of all k into y
                            py = psB.tile([128, K, 2, FO], fp32, tag="pb",
                                          name=f"py{b}")
                            nc.tensor.matmul(
                                py[:, :, :, :],
                                x_sb[:, j, cn * 128:(cn + 1) * 128],
                                th2_sb[:],
                                start=True, stop=True,
                            )
                            dst = y_sb[:, cn, :, tl0:tl0 + 2, :]
                            if (j + cn) % 2 == 0:
                                nc.vector.tensor_copy(dst, py[:, :, :, :])
                            else:
                                nc.scalar.copy(dst, py[:, :, :, :])

                def product():
                    att_c, att_r, p_sb = st['att_c'], st['att_r'], st['p']
                    for cn in range(NCH):
                        for s in range(2):
                            pp = psA.tile([128, 512], fp32, tag="big",
                                          name=f"pp{b}")
                            nc.tensor.matmul(
                                pp[:, :],
                                att_c[0:T, cn * 128:(cn + 1) * 128],
                                att_r[:, s * 512:(s + 1) * 512],
                                start=True, stop=True,
                            )
                            tmp = wpool.tile([128, 512], bf, tag="tmp",
                                             name=f"tmp{b}")
                            nc.vector.tensor_add(
                                tmp[:], pp[:],
                                bs_sb[:, cn, s * 512:(s + 1) * 512])
                            nc.scalar.activation(
                                p_sb[:, cn, s * 512:(s + 1) * 512], tmp[:],
                                AF.Tanh, scale=0.5)

                def sphase():
                    e_sb, p_sb, a_sb = st['e'], st['p'], st['a']
                    for ic in range(NCH):
                        for s in range(2):
                            ps = psA.tile([128, 512], fp32, tag="big",
                                          name=f"ps{b}")
                            for kc in range(NCH):
                                nc.tensor.matmul(
                                    ps[:, :],
                                    vsT_sb[:, kc, ic * 128:(ic + 1) * 128],
                                    p_sb[:, kc, s * 512:(s + 1) * 512],
                                    start=(kc == 0), stop=(kc == NCH - 1),
                                )
                            nc.scalar.activation(
                                e_sb[:, ic, s * 512:(s + 1) * 512], ps[:],
                                AF.Exp, scale=0.5,
                                bias=hrow_sb[:, ic:ic + 1],
                            )
                        # lower-half A-muls inline; upper halves deferred so
                        # the next batch's conv mc 0-3 can start sooner
                        for k in range(K):
                            ch = wpool.tile([128, 512], bf, tag="cheb",
                                            bufs=6, name=f"ch{b}")
                            nc.sync.dma_start(ch[:], cheb_d[k, ic, :, 0:512])
                            nc.vector.tensor_mul(a_sb[:, k, ic, 0:512],
                                                 ch[:], e_sb[:, ic, 0:512])

                def amul_h1():
                    e_sb, a_sb = st['e'], st['a']
                    for ic in range(NCH):
                        for k in range(K):
                            ch = wpool.tile([128, 512], bf, tag="cheb",
                                            bufs=6, name=f"ch1{b}")
                            nc.sync.dma_start(ch[:],
                                              cheb_d[k, ic, :, 512:1024])
                            nc.vector.tensor_mul(a_sb[:, k, ic, 512:1024],
                                                 ch[:], e_sb[:, ic, 512:1024])

                def rt():
                    e_sb, rT_sb = st['e'], st['rT']
                    pcs = [psA.tile([1, 512], fp32, tag="big",
                                    name=f"pc{b}_{s}") for s in range(2)]
                    for ic in range(NCH):
                        for s in range(2):
                            nc.tensor.matmul(
                                pcs[s][:, :],
                                ones_sb[:],
                                e_sb[:, ic, s * 512:(s + 1) * 512],
                                start=(ic == 0), stop=(ic == NCH - 1),
                            )
                    csum_sb = wpool.tile([1, N], fp32, tag="csum_s", bufs=1,
                                         name=f"cs{b}")
                    for s in range(2):
                        nc.scalar.copy(csum_sb[:, s * 512:(s + 1) * 512],
                                       pcs[s][:])
                    prt = psA.tile([128, NCH], fp32, tag="big",
                                   name=f"prt{b}")
                    for c in range(NCH):
                        nc.tensor.matmul(
                            prt[:, c:c + 1],
                            csum_sb[:, c * 128:(c + 1) * 128],
                            one1_sb[:],
                            start=True, stop=True,
                        )
                    nc.vector.reciprocal(rT_sb[:], prt[:])

                def conv(g):
                    a_sb, rT_sb, y_sb = st['a'], st['rT'], st['ys'][g]
                    for mc in range(NCH):
                        po = psB.tile([128, TT, FO], fp32, tag="pb",
                                      name=f"po{b}")
                        nmm = 0
                        for k in range(K):
                            for cn in range(NCH):
                                nc.tensor.matmul(
                                    po[:, :, :],
                                    a_sb[:, k, cn, mc * 128:(mc + 1) * 128],
                                    y_sb[:, cn, k, :, :],
                                    start=(nmm == 0),
                                    stop=(nmm == K * NCH - 1),
                                )
                                nmm += 1
                        stg = wpool.tile([128, FO, TT], bf, tag="stage",
                                         name=f"st{b}")
                        nc.scalar.activation(
                            stg[:],
                            po[:, :, :].rearrange("p t o -> p o t"),
                            AF.Relu,
                            scale=rT_sb[:, mc:mc + 1],
                        )
                        nc.sync.dma_start(out_d[b, g, mc], stg[:])

                st['head'] = head
                st['build_y'] = build_y
                st['product'] = product
                st['sphase'] = sphase
                st['amul_h1'] = amul_h1
                st['rt'] = rt
                st['conv'] = conv
                return st

            # ---- software-pipelined emission over the two batches ----
            b0 = make_batch(0)
            b0['head']()
            b0['build_y'](0)
            b0['product']()
            b0['build_y'](1)
            b0['sphase']()
            b0['rt']()
            b0['amul_h1']()
            b0['conv'](0)
            b0['build_y'](2)

            b1 = make_batch(1)
            b1['head']()
            b0['conv'](1)
            b1['build_y'](0)
            b1['product']()
            b0['conv'](2)
            b1['build_y'](1)
            b1['sphase']()
            b1['rt']()
            b1['amul_h1']()
            b1['conv'](0)
            b1['build_y'](2)
            b1['conv'](1)
            b1['conv'](2)

    nc.compile()
    return nc


def _host_prep(x, W1, W2, W3, bs, Vs, cheb, Theta):
    x = np.asarray(x, np.float32)
    W1 = np.asarray(W1, np.float32)
    W2 = np.asarray(W2, np.float32)
    W3 = np.asarray(W3, np.float32)
    bs = np.asarray(bs, np.float32)
    Vs = np.asarray(Vs, np.float32)
    cheb = np.asarray(cheb, np.float32)
    Theta = np.asarray(Theta, np.float32)

    x_tf = np.ascontiguousarray(x.transpose(0, 3, 2, 1)).reshape(B, NJ, 128, N)
    x_tf = x_tf.astype(bf16)
    bs_t = bs[0].reshape(NCH, 128, N).astype(bf16)
    vs_t = np.ascontiguousarray(Vs.T).reshape(NCH, 128, N).astype(bf16)
    cheb_t = cheb.reshape(K, NCH, 128, N).astype(bf16)
    t_idx = np.arange(T * F) // F
    f_idx = np.arange(T * F) % F
    wl_flat = W1[t_idx][:, None] * W2[f_idx, :]
    wr_flat = np.zeros((T * F, T), np.float32)
    wr_flat[np.arange(T * F), t_idx] = W3[f_idx]
    wcat = np.concatenate([wl_flat, wr_flat], axis=1)
    wcat = wcat.reshape(NJ, 128, 2 * T).astype(bf16)
    # columns ordered (k, par, o) so the Y copy is a single strided op
    th2 = np.zeros((128, 2 * K * FO), np.float32)
    for par in range(2):
        for k in range(K):
            c0 = k * 2 * FO + par * FO
            th2[par * F:(par + 1) * F, c0:c0 + FO] = Theta[k]
    th2 = th2.astype(bf16)
    hrow = (0.5 * Vs.sum(axis=1)).astype(np.float32).reshape(NCH, 128, 1)
    return x_tf, bs_t, vs_t, cheb_t, wcat, th2, hrow


def kernel(x, W1, W2, W3, bs, Vs, cheb, Theta, _return_results=False,
           _trace=False):
    from concourse.bass_utils import run_bass_kernel_spmd

    x_tf, bs_t, vs_t, cheb_t, wcat, th2, hrow = _host_prep(
        x, W1, W2, W3, bs, Vs, cheb, Theta)

    if "nc" not in _CACHE:
        _CACHE["nc"] = _build_nc()
    nc = _CACHE["nc"]

    shared = dict(bs_t=bs_t, vs_t=vs_t, cheb_t=cheb_t, wcat=wcat,
                  th2=th2, hrow=hrow)
    in_maps = []
    for c in range(NCORES):
        m = dict(shared)
        m["x_tf"] = np.ascontiguousarray(x_tf[c * BPC:(c + 1) * BPC])
        in_maps.append(m)

    _CACHE["in_maps"] = in_maps
    kw = {"trace": True} if _trace else {}
    res = run_bass_kernel_spmd(nc, in_maps, list(range(NCORES)), **kw)
    outs = []
    for c in range(NCORES):
        o = res.results[c]["out"]  # (BPC, NG, NCH, 128, FO, TT)
        o = np.asarray(o, np.float32)
        o = o.transpose(0, 2, 3, 4, 1, 5).reshape(BPC, N, FO, T)
        outs.append(o)
    full = np.concatenate(outs, axis=0).astype(np.float32)
    if _return_results:
        return full, res
    return full


# revision 29
# speedup vs baseline: 1.1942x; 1.1942x over previous
"""Trainium2 Bass kernel for the MAMGCN encoder block.

Strategy: data-parallel over batch B=16 across 8 NeuronCores (2 batches/core).
Host-side prep (untimed): shard x, repack small weights, pre-transpose x to
(t*64+f, n) layout, cast matmul operands to bf16. Device does everything else:
spatial attention (two fused weight matmuls -> product -> tanh-sigmoid ->
Vs@P -> exp -> column softmax), Chebyshev graph conv with Theta folded in
(Y = X @ Theta2 block-diag), all matmuls in bf16 with fp32 PSUM accumulation.

v3: T in 3 groups of 8 (conv matmuls stream one full 512-row PSUM bank),
Y-build interleaved with attention phases (tensor stays dense, HAM warm),
Y PSUM->SBUF copies split across DVE and ACT, one shared 4-slot PSUM pool
for Y-build + conv, x loads issued ahead of the attention-constant loads,
and the two batches software-pipelined so batch 1's attention/S phases
overlap batch 0's graph conv.
"""
import numpy as np
import ml_dtypes

B, N, F, T, K, FO = 16, 1024, 64, 24, 3, 64
NCORES = 8
BPC = B // NCORES          # batches per core
NCH = N // 128             # 8 partition chunks of N
NJ = (T * F) // 128        # 12 chunks of the tf dim
NG = 3                     # t-groups
TT = T // NG               # 8 t's per group
NJG = NJ // NG             # 4 tf-chunks per t-group
bf16 = ml_dtypes.bfloat16

_CACHE = {}


def _build_nc():
    import concourse.bacc as bacc
    import concourse.bass as bass
    import concourse.tile as tile
    import concourse.mybir as mybir

    fp32 = mybir.dt.float32
    bf = mybir.dt.bfloat16
    AF = mybir.ActivationFunctionType

    nc = bacc.Bacc(
        "TRN2", target_bir_lowering=False, debug=False,
        enable_asserts=True, num_devices=NCORES,
    )

    # ---- DRAM I/O ----
    x_d = nc.dram_tensor("x_tf", [BPC, NJ, 128, N], bf, kind="ExternalInput")
    bs_d = nc.dram_tensor("bs_t", [NCH, 128, N], bf, kind="ExternalInput")
    vs_d = nc.dram_tensor("vs_t", [NCH, 128, N], bf, kind="ExternalInput")
    cheb_d = nc.dram_tensor("cheb_t", [K, NCH, 128, N], bf, kind="ExternalInput")
    wcat_d = nc.dram_tensor("wcat", [NJ, 128, 2 * T], bf, kind="ExternalInput")
    th2_d = nc.dram_tensor("th2", [128, 2 * K * FO], bf, kind="ExternalInput")
    hrow_d = nc.dram_tensor("hrow", [NCH, 128, 1], fp32, kind="ExternalInput")
    # out[b, group, mchunk, p, o, tl]
    out_d = nc.dram_tensor("out", [BPC, NG, NCH, 128, FO, TT], bf,
                           kind="ExternalOutput")

    with tile.TileContext(nc) as tc:
        with (
            tc.tile_pool(name="const", bufs=1) as cpool,
            tc.tile_pool(name="work", bufs=2) as wpool,
            tc.tile_pool(name="big", bufs=1) as bpool,
            tc.tile_pool(name="ypool", bufs=2) as ypool,
            tc.tile_pool(name="psA", bufs=2, space="PSUM") as psA,
            tc.tile_pool(name="psB", bufs=6, space="PSUM") as psB,
        ):
            # ---- constants needed first (attention + Y-build) ----
            wcat_sb = cpool.tile([128, NJ, 2 * T], bf, tag="wcat")
            th2_sb = cpool.tile([128, 2 * K * FO], bf, tag="th2")
            vsT_sb = cpool.tile([128, NCH, N], bf, tag="vsT")
            bs_sb = cpool.tile([128, NCH, N], bf, tag="bs")
            hrow_sb = cpool.tile([128, NCH], fp32, tag="hrow")
            ones_sb = cpool.tile([128, 1], bf, tag="ones")
            one1_sb = cpool.tile([1, 1], fp32, tag="one1")
            nc.gpsimd.memset(ones_sb[:], 1.0)
            nc.gpsimd.memset(one1_sb[:], 1.0)

            def make_batch(b):
                st = {}

                def head():
                    st['x'] = x_sb = bpool.tile([128, NJ, N], bf, tag="x",
                                                name=f"x{b}")
                    st['e'] = bpool.tile([128, NCH, N], bf, tag="e",
                                         name=f"e{b}")
                    st['p'] = bpool.tile([128, NCH, N], bf, tag="p",
                                         name=f"p{b}")
                    st['a'] = bpool.tile([128, K, NCH, N], bf, tag="a",
                                         name=f"a{b}")
                    st['rT'] = bpool.tile([128, NCH], fp32, tag="rT",
                                          name=f"rT{b}")
                    st['ys'] = []
                    for j in range(NJ):
                        if b == 0:
                            nc.sync.dma_start(wcat_sb[:, j, :], wcat_d[j])
                        nc.sync.dma_start(x_sb[:, j, :], x_d[b, j])
                        if b == 0 and j == 1:
                            nc.sync.dma_start(th2_sb[:], th2_d[:])
                    if b == 0:
                        # attention constants follow the x load in queue order
                        for c in range(NCH):
                            nc.sync.dma_start(bs_sb[:, c, :], bs_d[c])
                        for c in range(NCH):
                            nc.sync.dma_start(vsT_sb[:, c, :], vs_d[c])
                            nc.sync.dma_start(hrow_sb[:, c:c + 1], hrow_d[c])
                    # attention pre-reductions (one pass over x)
                    st['att_c'] = att_c = wpool.tile([2 * T, N], bf,
                                                     tag="attc", bufs=1,
                                                     name=f"attc{b}")
                    st['att_r'] = att_r = wpool.tile([T, N], bf, tag="attr",
                                                     bufs=1, name=f"attr{b}")
                    pas = [psA.tile([2 * T, 512], fp32, tag="big",
                                    name=f"pa{b}_{s}") for s in range(2)]
                    for j in range(NJ):
                        for s in range(2):
                            nc.tensor.matmul(
                                pas[s][:, :],
                                wcat_sb[:, j, :],
                                x_sb[:, j, s * 512:(s + 1) * 512],
                                start=(j == 0), stop=(j == NJ - 1),
                            )
                    for s in range(2):
                        nc.scalar.copy(att_c[:, s * 512:(s + 1) * 512],
                                       pas[s][:])
                    # shift rows 24..47 down to partitions 0..23
                    nc.sync.dma_start(att_r[:], att_c[T:2 * T, :])

                def build_y(g):
                    x_sb = st['x']
                    y_sb = ypool.tile([128, NCH, K, TT, FO], bf, tag="y",
                                      name=f"y{b}_{g}")
                    st['ys'].append(y_sb)
                    for j in range(NJG * g, NJG * (g + 1)):
                        tl0 = 2 * (j - NJG * g)
                        for cn in range(NCH):
                            # py layout (k, par, o); one strided copy moves
                            # both t's of all k into y
                            py = psB.tile([128, K, 2, FO], fp32, tag="pb",
                                          name=f"py{b}")
                            nc.tensor.matmul(
                                py[:, :, :, :],
                                x_sb[:, j, cn * 128:(cn + 1) * 128],
                                th2_sb[:],
                                start=True, stop=True,
                            )
                            dst = y_sb[:, cn, :, tl0:tl0 + 2, :]
                            if (j + cn) % 2 == 0:
                                nc.vector.tensor_copy(dst, py[:, :, :, :])
                            else:
                                nc.scalar.copy(dst, py[:, :, :, :])

                def product():
                    att_c, att_r, p_sb = st['att_c'], st['att_r'], st['p']
                    for cn in range(NCH):
                        for s in range(2):
                            pp = psA.tile([128, 512], fp32, tag="big",
                                          name=f"pp{b}")
                            nc.tensor.matmul(
                                pp[:, :],
                                att_c[0:T, cn * 128:(cn + 1) * 128],
                                att_r[:, s * 512:(s + 1) * 512],
                                start=True, stop=True,
                            )
                            tmp = wpool.tile([128, 512], bf, tag="tmp",
                                             name=f"tmp{b}")
                            nc.vector.tensor_add(
                                tmp[:], pp[:],
                                bs_sb[:, cn, s * 512:(s + 1) * 512])
                            nc.scalar.activation(
                                p_sb[:, cn, s * 512:(s + 1) * 512], tmp[:],
                                AF.Tanh, scale=0.5)

                def sphase():
                    e_sb, p_sb, a_sb = st['e'], st['p'], st['a']
                    for ic in range(NCH):
                        for s in range(2):
                            ps = psA.tile([128, 512], fp32, tag="big",
                                          name=f"ps{b}")
                            for kc in range(NCH):
                                nc.tensor.matmul(
                                    ps[:, :],
                                    vsT_sb[:, kc, ic * 128:(ic + 1) * 128],
                                    p_sb[:, kc, s * 512:(s + 1) * 512],
                                    start=(kc == 0), stop=(kc == NCH - 1),
                                )
                            nc.scalar.activation(
                                e_sb[:, ic, s * 512:(s + 1) * 512], ps[:],
                                AF.Exp, scale=0.5,
                                bias=hrow_sb[:, ic:ic + 1],
                            )
                        for k in range(K):
                            ch = wpool.tile([128, N], bf, tag="cheb", bufs=3,
                                            name=f"ch{b}")
                            nc.sync.dma_start(ch[:], cheb_d[k, ic])
                            nc.vector.tensor_mul(a_sb[:, k, ic, :], ch[:],
                                                 e_sb[:, ic, :])

                def rt():
                    e_sb, rT_sb = st['e'], st['rT']
                    pcs = [psA.tile([1, 512], fp32, tag="big",
                                    name=f"pc{b}_{s}") for s in range(2)]
                    for ic in range(NCH):
                        for s in range(2):
                            nc.tensor.matmul(
                                pcs[s][:, :],
                                ones_sb[:],
                                e_sb[:, ic, s * 512:(s + 1) * 512],
                                start=(ic == 0), stop=(ic == NCH - 1),
                            )
                    csum_sb = wpool.tile([1, N], fp32, tag="csum_s", bufs=1,
                                         name=f"cs{b}")
                    for s in range(2):
                        nc.scalar.copy(csum_sb[:, s * 512:(s + 1) * 512],
                                       pcs[s][:])
                    prt = psA.tile([128, NCH], fp32, tag="big",
                                   name=f"prt{b}")
                    for c in range(NCH):
                        nc.tensor.matmul(
                            prt[:, c:c + 1],
                            csum_sb[:, c * 128:(c + 1) * 128],
                            one1_sb[:],
                            start=True, stop=True,
                        )
                    nc.vector.reciprocal(rT_sb[:], prt[:])

                def conv(g):
                    a_sb, rT_sb, y_sb = st['a'], st['rT'], st['ys'][g]
                    for mc in range(NCH):
                        po = psB.tile([128, TT, FO], fp32, tag="pb",
                                      name=f"po{b}")
                        nmm = 0
                        for k in range(K):
                            for cn in range(NCH):
                                nc.tensor.matmul(
                                    po[:, :, :],
                                    a_sb[:, k, cn, mc * 128:(mc + 1) * 128],
                                    y_sb[:, cn, k, :, :],
                                    start=(nmm == 0),
                                    stop=(nmm == K * NCH - 1),
                                )
                                nmm += 1
                        stg = wpool.tile([128, FO, TT], bf, tag="stage",
                                         name=f"st{b}")
                        nc.scalar.activation(
                            stg[:],
                            po[:, :, :].rearrange("p t o -> p o t"),
                            AF.Relu,
                            scale=rT_sb[:, mc:mc + 1],
                        )
                        nc.sync.dma_start(out_d[b, g, mc], stg[:])

                st['head'] = head
                st['build_y'] = build_y
                st['product'] = product
                st['sphase'] = sphase
                st['rt'] = rt
                st['conv'] = conv
                return st

            # ---- software-pipelined emission over the two batches ----
            b0 = make_batch(0)
            b0['head']()
            b0['build_y'](0)
            b0['product']()
            b0['build_y'](1)
            b0['sphase']()
            b0['rt']()
            b0['conv'](0)
            b0['build_y'](2)

            b1 = make_batch(1)
            b1['head']()
            b0['conv'](1)
            b1['build_y'](0)
            b1['product']()
            b0['conv'](2)
            b1['build_y'](1)
            b1['sphase']()
            b1['rt']()
            b1['conv'](0)
            b1['build_y'](2)
            b1['conv'](1)
            b1['conv'](2)

    nc.compile()
    return nc


def _host_prep(x, W1, W2, W3, bs, Vs, cheb, Theta):
    x = np.asarray(x, np.float32)
    W1 = np.asarray(W1, np.float32)
    W2 = np.asarray(W2, np.float32)
    W3 = np.asarray(W3, np.float32)
    bs = np.asarray(bs, np.float32)
    Vs = np.asarray(Vs, np.float32)
    cheb = np.asarray(cheb, np.float32)
    Theta = np.asarray(Theta, np.float32)

    x_tf = np.ascontiguousarray(x.transpose(0, 3, 2, 1)).reshape(B, NJ, 128, N)
    x_tf = x_tf.astype(bf16)
    bs_t = bs[0].reshape(NCH, 128, N).astype(bf16)
    vs_t = np.ascontiguousarray(Vs.T).reshape(NCH, 128, N).astype(bf16)
    cheb_t = cheb.reshape(K, NCH, 128, N).astype(bf16)
    t_idx = np.arange(T * F) // F
    f_idx = np.arange(T * F) % F
    wl_flat = W1[t_idx][:, None] * W2[f_idx, :]
    wr_flat = np.zeros((T * F, T), np.float32)
    wr_flat[np.arange(T * F), t_idx] = W3[f_idx]
    wcat = np.concatenate([wl_flat, wr_flat], axis=1)
    wcat = wcat.reshape(NJ, 128, 2 * T).astype(bf16)
    # columns ordered (k, par, o) so the Y copy is a single strided op
    th2 = np.zeros((128, 2 * K * FO), np.float32)
    for par in range(2):
        for k in range(K):
            c0 = k * 2 * FO + par * FO
            th2[par * F:(par + 1) * F, c0:c0 + FO] = Theta[k]
    th2 = th2.astype(bf16)
    hrow = (0.5 * Vs.sum(axis=1)).astype(np.float32).reshape(NCH, 128, 1)
    return x_tf, bs_t, vs_t, cheb_t, wcat, th2, hrow


def kernel(x, W1, W2, W3, bs, Vs, cheb, Theta, _return_results=False,
           _trace=False):
    from concourse.bass_utils import run_bass_kernel_spmd

    x_tf, bs_t, vs_t, cheb_t, wcat, th2, hrow = _host_prep(
        x, W1, W2, W3, bs, Vs, cheb, Theta)

    if "nc" not in _CACHE:
        _CACHE["nc"] = _build_nc()
    nc = _CACHE["nc"]

    shared = dict(bs_t=bs_t, vs_t=vs_t, cheb_t=cheb_t, wcat=wcat,
                  th2=th2, hrow=hrow)
    in_maps = []
    for c in range(NCORES):
        m = dict(shared)
        m["x_tf"] = np.ascontiguousarray(x_tf[c * BPC:(c + 1) * BPC])
        in_maps.append(m)

    _CACHE["in_maps"] = in_maps
    kw = {"trace": True} if _trace else {}
    res = run_bass_kernel_spmd(nc, in_maps, list(range(NCORES)), **kw)
    outs = []
    for c in range(NCORES):
        o = res.results[c]["out"]  # (BPC, NG, NCH, 128, FO, TT)
        o = np.asarray(o, np.float32)
        o = o.transpose(0, 2, 3, 4, 1, 5).reshape(BPC, N, FO, T)
        outs.append(o)
    full = np.concatenate(outs, axis=0).astype(np.float32)
    if _return_results:
        return full, res
    return full
